# revision 16
# baseline (speedup 1.0000x reference)
"""DeepseekV2 MLA attention on 8 Trainium2 NeuronCores.

Sharding (uniform SPMD, no control divergence):
- A-projection, q-up-projection, final W_O: token-sharded (core c owns
  tokens [256c, 256c+256)).
- Attention (absorbed MLA over the compressed KV latent): head-sharded
  (core c owns heads {2c, 2c+1}).
- Three collectives connect the shardings: AllGather of the kv latent
  (feature-major, bf16), AllToAll of q^T (token->head resharding),
  AllToAll of normalized o^T (head->token resharding).

All matmuls run in bf16 with fp32 PSUM accumulation. RMSNorm weights are
folded into the adjacent weight matrices on the host. Softmax runs
unnormalized (logits are small by construction: std ~0.7) with the
denominator obtained by appending a ones-column to V; normalization is a
per-partition scale on the token-major attention output.

v2: all weight streams are host-packed into large contiguous p-major
blocks so each DMA moves 0.6-2.6 MB (the sync queue serializes DMA
issue at ~600ns each, capping many-small-DMA phases at ~100-200 GB/s).
RMSNorm/rope consume the A-proj PSUM directly; final output writes go
out on the gpsimd queue so they cannot stall w_o weight prefetch.
"""

import os
import sys

for _p in ("/opt/trn_rl_repo", "/root/.axon_site", "/root/.axon_site/_ro/trn_rl_repo",
           "/root/.axon_site/_ro/pypackages"):
    if os.path.isdir(_p) and _p not in sys.path:
        sys.path.insert(0, _p)

import numpy as np
import ml_dtypes

import concourse.bass as bass
import concourse.tile as tile
from concourse import bacc, mybir
from concourse.bass_utils import run_bass_kernel_spmd
from concourse.masks import make_identity

# Problem constants (hardcoded per contract)
T, HID, H = 2048, 5120, 16
DN, DR, DV = 128, 64, 128
QL, KVL = 1536, 512
EPS = 1e-6
THETA = 10000.0
SCALE = (DN + DR) ** -0.5

NCORES = 8
TLOC = T // NCORES          # 256 tokens per core
HLOC = H // NCORES          # 2 heads per core
MCH = TLOC // 128           # 2 token chunks of 128
KD = HID // 128             # 40 contraction chunks for A-proj
QKD = QL // 128             # 12 contraction chunks for q-up
LC = KVL // 128             # 4 latent chunks
NQB = T // 128              # 16 query/key blocks of 128
DQK = DN + DR               # 192
DVE_ = DV + 1               # 129: extra ones-column for softmax denominator
NGQ = H * DQK // 512        # 6 q-up output groups of 512
NHT = HID // 512            # 10 W_O output groups of 512

# A-proj column groups: (g0, gw, n_halves) in processing order
# (kv latent first so the AllGather chain launches early, then rope
# cols, then the three q-latent groups).
AGROUPS = [(QL, KVL, 4), (QL + KVL, DR, 1), (0, 512, 2), (512, 512, 2),
           (1024, 512, 2)]

BF = mybir.dt.bfloat16
F32 = mybir.dt.float32

_NC_CACHE = None
_last_in_maps = None


def _rope_pair(nc, pool, x_pairs, cos, sin, out_pairs, shape, dt=F32):
    """Interleaved rope: out1 = x1*cos - x2*sin ; out2 = x2*cos + x1*sin."""
    x1, x2 = x_pairs[:, 0], x_pairs[:, 1]
    o1, o2 = out_pairs[:, 0], out_pairs[:, 1]
    tm1 = pool.tile([128] + shape, dt, tag="rope_tm1", name="rope_tm1")
    tm2 = pool.tile([128] + shape, dt, tag="rope_tm2", name="rope_tm2")
    tm3 = pool.tile([128] + shape, dt, tag="rope_tm3", name="rope_tm3")
    nc.vector.tensor_mul(tm1[:], x1, cos)
    nc.vector.tensor_mul(tm2[:], x2, sin)
    nc.vector.tensor_mul(tm3[:], x1, sin)
    nc.vector.tensor_sub(o1, tm1[:], tm2[:])
    nc.vector.tensor_mul(tm1[:], x2, cos)
    nc.vector.tensor_add(o2, tm1[:], tm3[:])


def build_nc():
    nc = bacc.Bacc("TRN2", target_bir_lowering=False, debug=False,
                   num_devices=NCORES)

    hT = nc.dram_tensor("hT", [KD * 128 * TLOC], BF, kind="ExternalInput")
    wf = nc.dram_tensor("wf", [HID * (QL + KVL + DR)], BF, kind="ExternalInput")
    wqb = nc.dram_tensor("wqb", [QL * H * DQK], BF, kind="ExternalInput")
    cs = nc.dram_tensor("cs", [TLOC, DR], F32, kind="ExternalInput")
    wkcT = nc.dram_tensor("wkcT", [HLOC, DN, KVL], BF, kind="ExternalInput")
    wvc = nc.dram_tensor("wvc", [KVL, HLOC * DV], BF, kind="ExternalInput")
    wo = nc.dram_tensor("wo", [H * DV * HID], BF, kind="ExternalInput")
    cmask = nc.dram_tensor("cmask", [128, HLOC, 128], BF, kind="ExternalInput")
    out = nc.dram_tensor("out", [TLOC, HID], F32, kind="ExternalOutput")

    RG = [list(range(NCORES))]

    with tile.TileContext(nc) as tc:
        consts_cm = tc.tile_pool(name="consts", bufs=1)
        consts = consts_cm.__enter__()
        dram_cm = tc.tile_pool(name="dram", bufs=1, space="DRAM")
        dram = dram_cm.__enter__()
        ps_mm_cm = tc.tile_pool(name="ps_mm", bufs=4, space="PSUM")
        ps_mm = ps_mm_cm.__enter__()
        ps_tr_cm = tc.tile_pool(name="ps_tr", bufs=2, space="PSUM")
        ps_tr = ps_tr_cm.__enter__()
        attnkv_cm = tc.tile_pool(name="attnkv", bufs=1)
        attnkv = attnkv_cm.__enter__()

        ident = consts.tile([128, 128], BF, name="ident")
        make_identity(nc, ident[:])
        cmask_sb = consts.tile([128, HLOC, 128], BF, name="cmask_sb")
        wkc_sb = consts.tile([128, HLOC, KVL], BF, name="wkc_sb")
        eps_sb = consts.tile([128, 1], F32, name="eps_sb")
        nc.vector.memset(eps_sb[:], float(EPS))
        wvc_sb = consts.tile([128, LC, HLOC * DV], BF, name="wvc_sb")

        # collective DRAM tiles
        ag_in = dram.tile([KVL + DR, TLOC], BF, name="ag_in")
        ag_out = dram.tile([NCORES, KVL + DR, TLOC], BF, addr_space="Shared",
                           name="ag_out")
        a2aq_in = dram.tile([NCORES, HLOC, DQK, TLOC], BF, name="a2aq_in")
        a2aq_out = dram.tile([NCORES, HLOC, DQK, TLOC], BF, name="a2aq_out")
        a2ao_in = dram.tile([NCORES, HLOC, DV, TLOC], BF, name="a2ao_in")
        a2ao_out = dram.tile([NCORES, HLOC, DV, TLOC], BF, name="a2ao_out")

        # ---------------- Stages 1-3: token-sharded projections -----------
        early_cm = tc.tile_pool(name="early", bufs=1)
        early = early_cm.__enter__()
        tmp_cm = tc.tile_pool(name="tmp", bufs=1)
        tmp = tmp_cm.__enter__()
        wfpool_cm = tc.tile_pool(name="wfpool", bufs=2)
        wfpool = wfpool_cm.__enter__()

        hT_sb = early.tile([128, KD, TLOC], BF, name="hT_sb")
        KH = KD // 2
        for h in range(2):
            nc.sync.dma_start(
                out=hT_sb[:, h * KH:(h + 1) * KH, :],
                in_=hT[h * 128 * KH * TLOC:(h + 1) * 128 * KH * TLOC]
                    .rearrange("(p x) -> p x", p=128))
        cs_sb = early.tile([128, MCH, DR], F32, name="cs_sb")
        nc.sync.dma_start(out=cs_sb[:],
                          in_=cs[:, :].rearrange("(m p) d -> p m d", p=128))

        wf_off = [0]

        def aproj_group(g0, gw, nh):
            """A-proj over one column group; returns per-m psum tiles."""
            kper = KD // nh
            pss = [ps_mm.tile([128, gw], F32, tag="mm", name="aproj_ps")
                   for _ in range(MCH)]
            for hh in range(nh):
                wf_t = wfpool.tile([128, kper * gw], BF, tag="wf_t", name="wf_t")
                nc.sync.dma_start(
                    out=wf_t[:],
                    in_=wf[wf_off[0]:wf_off[0] + 128 * kper * gw]
                        .rearrange("(p w) -> p w", p=128))
                wf_off[0] += 128 * kper * gw
                for kk in range(kper):
                    k = hh * kper + kk
                    for m in range(MCH):
                        nc.tensor.matmul(
                            pss[m][:], hT_sb[:, k, m * 128:(m + 1) * 128],
                            wf_t[:, kk * gw:(kk + 1) * gw],
                            start=(k == 0), stop=(k == KD - 1))
            return pss

        pss_kv = aproj_group(*AGROUPS[0])
        pss_pe = aproj_group(*AGROUPS[1])

        # ---------------- Stage 2: kv latent + rope + AllGather -----------
        kvlat_bf = early.tile([128, MCH, KVL], BF, name="kvlat_bf")
        kpe_bf = early.tile([128, MCH, DR], BF, name="kpe_bf")
        agin_sb = early.tile([128, LC, MCH, 128], BF, name="agin_sb")
        agpe_sb = early.tile([64, MCH, 128], BF, name="agpe_sb")

        for m in range(MCH):
            sq = tmp.tile([128, KVL], F32, tag="sq_kv", name="sq_kv")
            ssum = tmp.tile([128, 1], F32, tag="ssum_kv", name="ssum_kv")
            nc.scalar.activation(sq[:], pss_kv[m][:],
                                 mybir.ActivationFunctionType.Square,
                                 accum_out=ssum[:])
            rstd = tmp.tile([128, 1], F32, tag="rstd_kv", name="rstd_kv")
            nc.scalar.activation(rstd[:], ssum[:],
                                 mybir.ActivationFunctionType.Sqrt,
                                 bias=eps_sb[:], scale=1.0 / KVL)
            rinv = tmp.tile([128, 1], F32, tag="rinv_kv", name="rinv_kv")
            nc.vector.reciprocal(rinv[:], rstd[:])
            nc.vector.tensor_scalar_mul(kvlat_bf[:, m], in0=pss_kv[m][:],
                                        scalar1=rinv[:])
            kv_pairs = pss_pe[m][:].rearrange("p (i two) -> p two i", two=2)
            out_pairs = kpe_bf[:, m].rearrange("p (i two) -> p two i", two=2)
            _rope_pair(nc, tmp, kv_pairs,
                       cs_sb[:, m, :DR // 2], cs_sb[:, m, DR // 2:],
                       out_pairs, [DR // 2])
            for lc in range(LC):
                pt = ps_tr.tile([128, 128], BF, tag="tr", name="pt_tr")
                nc.tensor.transpose(pt[:], kvlat_bf[:, m, lc * 128:(lc + 1) * 128],
                                    ident[:])
                nc.vector.tensor_copy(agin_sb[:, lc, m, :], pt[:])
            ptp = ps_tr.tile([64, 128], BF, tag="tr", name="ptp_tr")
            nc.tensor.transpose(ptp[:], kpe_bf[:, m], ident[:])
            nc.vector.tensor_copy(agpe_sb[:, m, :], ptp[:])

        nc.gpsimd.dma_start(
            out=ag_in[:KVL, :].rearrange("(c p) m -> p c m", p=128)
                              .rearrange("p c (m t) -> p c m t", m=MCH),
            in_=agin_sb[:])
        nc.gpsimd.dma_start(
            out=ag_in[KVL:, :].rearrange("p (m t) -> p m t", m=MCH),
            in_=agpe_sb[:])
        nc.gpsimd.collective_compute(
            "AllGather", mybir.AluOpType.bypass, replica_groups=RG,
            ins=[ag_in.opt()], outs=[ag_out.opt()])

        # attention-phase constants: loaded off the startup critical path
        nc.sync.dma_start(out=cmask_sb[:], in_=cmask[:, :, :])
        nc.sync.dma_start(out=wkc_sb[:], in_=wkcT[:, :, :].rearrange("h d l -> d h l"))
        nc.sync.dma_start(out=wvc_sb[:],
                          in_=wvc[:, :].rearrange("(c p) v -> p c v", p=128))

        # ---------------- Stage 3: q path ---------------------------------
        qkv_q = early.tile([128, MCH, QL], F32, name="qkv_q")
        qg_ss = []
        for gi in range(3):
            pss = aproj_group(*AGROUPS[2 + gi])
            for m in range(MCH):
                sq2 = tmp.tile([128, 512], F32, tag="sq_q", name="sq_q")
                ssum2 = tmp.tile([128, 1], F32, tag=f"ssum_q{gi}m{m}",
                                 name="ssum_q")
                nc.scalar.activation(sq2[:], pss[m][:],
                                     mybir.ActivationFunctionType.Square,
                                     accum_out=ssum2[:])
                nc.vector.tensor_copy(qkv_q[:, m, gi * 512:(gi + 1) * 512],
                                      pss[m][:])
                qg_ss.append(ssum2)
        wfpool_cm.__exit__(None, None, None)

        qan_bf = early.tile([128, MCH, QL], BF, name="qan_bf")
        for m in range(MCH):
            st = tmp.tile([128, 1], F32, tag=f"stot{m}", name="stot")
            nc.vector.tensor_add(st[:], qg_ss[m][:], qg_ss[2 + m][:])
            nc.vector.tensor_add(st[:], st[:], qg_ss[4 + m][:])
            rstd2 = tmp.tile([128, 1], F32, tag="rstd_q", name="rstd_q")
            nc.scalar.activation(rstd2[:], st[:],
                                 mybir.ActivationFunctionType.Sqrt,
                                 bias=eps_sb[:], scale=1.0 / QL)
            rinv2 = tmp.tile([128, 1], F32, tag="rinv_q", name="rinv_q")
            nc.vector.reciprocal(rinv2[:], rstd2[:])
            nc.vector.tensor_scalar_mul(qan_bf[:, m], in0=qkv_q[:, m],
                                        scalar1=rinv2[:])

        qanT_sb = early.tile([128, QKD, TLOC], BF, name="qanT_sb")
        for m in range(MCH):
            for kc in range(QKD):
                pt = ps_tr.tile([128, 128], BF, tag="tr", name="pt_tr")
                nc.tensor.transpose(pt[:], qan_bf[:, m, kc * 128:(kc + 1) * 128],
                                    ident[:])
                nc.vector.tensor_copy(qanT_sb[:, kc, m * 128:(m + 1) * 128], pt[:])

        # q-up: one 1.57MB weight DMA per 512-col output group
        q_bf = early.tile([128, MCH, H, DQK], BF, name="q_bf")
        wqbpool_cm = tc.tile_pool(name="wqbpool", bufs=2)
        wqbpool = wqbpool_cm.__enter__()
        for ng in range(NGQ):
            wqb_t = wqbpool.tile([128, QKD * 512], BF, tag="wqb_t", name="wqb_t")
            _qo = ng * QKD * 128 * 512
            nc.sync.dma_start(
                out=wqb_t[:],
                in_=wqb[_qo:_qo + QKD * 128 * 512].rearrange("(p w) -> p w", p=128))
            psq = [ps_mm.tile([128, 512], F32, tag="mm", name="qup_ps")
                   for _ in range(MCH)]
            for kc in range(QKD):
                for m in range(MCH):
                    nc.tensor.matmul(psq[m][:], qanT_sb[:, kc, m * 128:(m + 1) * 128],
                                     wqb_t[:, kc * 512:(kc + 1) * 512],
                                     start=(kc == 0), stop=(kc == QKD - 1))
            for m in range(MCH):
                dst = q_bf[:, m].rearrange("p h d -> p (h d)")[:,
                                                              ng * 512:(ng + 1) * 512]
                if ng % 2 == 0:
                    nc.scalar.copy(dst, psq[m][:])
                else:
                    nc.vector.tensor_copy(dst, psq[m][:])
        wqbpool_cm.__exit__(None, None, None)

        # K/V prep for attention: depends only on the AllGather, so load
        # via the (now idle) sync queue and fill the tensor queue while
        # the q AllToAll input is still being built / in flight.
        kT_sb = attnkv.tile([128, LC, T], BF, name="kT_sb")
        for lc in range(LC):
            nc.sync.dma_start(
                out=kT_sb[:, lc].rearrange("p (s t) -> p s t", s=NCORES),
                in_=ag_out[:, lc * 128:(lc + 1) * 128, :].rearrange("s p t -> p s t"))
        kTpe_sb = attnkv.tile([64, T], BF, name="kTpe_sb")
        nc.sync.dma_start(
            out=kTpe_sb[:].rearrange("p (s t) -> p s t", s=NCORES),
            in_=ag_out[:, KVL:, :].rearrange("s p t -> p s t"))
        v_sb = attnkv.tile([128, NQB, HLOC, DVE_], BF, name="v_sb")
        nc.vector.memset(v_sb[:, :, :, DV:], 1.0)
        for tcb in range(NQB):
            pv = ps_mm.tile([128, HLOC * DV], F32, tag="mm", name="pv_ps")
            for lc in range(LC):
                nc.tensor.matmul(pv[:], kT_sb[:, lc, tcb * 128:(tcb + 1) * 128],
                                 wvc_sb[:, lc, :], start=(lc == 0),
                                 stop=(lc == LC - 1))
            nc.vector.tensor_copy(
                v_sb[:, tcb, :, :DV],
                pv[:].rearrange("p (h v) -> p h v", h=HLOC))

        # rope on q_pe: batched over all heads, bf16 in-place on q_bf
        # (cos/sin broadcast across heads via a zero-stride AP dim)
        cs_bf = early.tile([128, MCH, DR], BF, name="cs_bf")
        nc.vector.tensor_copy(cs_bf[:], cs_sb[:])

        def _bcast_h(apx):
            return bass.AP(tensor=apx.tensor, offset=apx.offset,
                           ap=[apx.ap[0], [0, H], apx.ap[-1]])

        for m in range(MCH):
            q_pairs = q_bf[:, m, :, DN:].rearrange("p h (i two) -> p two h i", two=2)
            _rope_pair(nc, tmp, q_pairs,
                       _bcast_h(cs_bf[:, m, :DR // 2]),
                       _bcast_h(cs_bf[:, m, DR // 2:]),
                       q_pairs, [H, DR // 2], dt=BF)

        aq0_sb = early.tile([128, H, MCH, 128], BF, name="aq0_sb")
        aq1_sb = early.tile([64, H, MCH, 128], BF, name="aq1_sb")
        for m in range(MCH):
            for h in range(H):
                pt0 = ps_tr.tile([128, 128], BF, tag="tr", name="pt0_tr")
                nc.tensor.transpose(pt0[:], q_bf[:, m, h, :DN], ident[:])
                nc.vector.tensor_copy(aq0_sb[:, h, m, :], pt0[:])
                pt1 = ps_tr.tile([64, 128], BF, tag="tr", name="pt1_tr")
                nc.tensor.transpose(pt1[:], q_bf[:, m, h, DN:], ident[:])
                nc.vector.tensor_copy(aq1_sb[:, h, m, :], pt1[:])
        _aqv = a2aq_in[:, :, :, :].rearrange("s hh d t -> (s hh) d t") \
                                  .rearrange("h d (m t) -> d h m t", m=MCH)
        nc.gpsimd.dma_start(out=_aqv[:DN], in_=aq0_sb[:])
        nc.gpsimd.dma_start(out=_aqv[DN:], in_=aq1_sb[:])
        nc.gpsimd.collective_compute(
            "AllToAll", mybir.AluOpType.bypass, replica_groups=RG,
            ins=[a2aq_in.opt()], outs=[a2aq_out.opt()])

        tmp_cm.__exit__(None, None, None)
        early_cm.__exit__(None, None, None)

        attn_cm = tc.tile_pool(name="attn", bufs=1)
        attn = attn_cm.__enter__()

        # ---------------- Stage 5: q^T for my heads -----------------------
        qT_sb = attn.tile([128, HLOC, T], BF, name="qT_sb")
        qTpe_sb = attn.tile([64, NQB, HLOC, 128], BF, name="qTpe_sb")
        for h in range(HLOC):
            nc.gpsimd.dma_start(
                out=qT_sb[:, h].rearrange("p (s t) -> p s t", s=NCORES),
                in_=a2aq_out[:, h, :DN, :].rearrange("s d t -> d s t"))
            for q2 in range(2):
                nc.gpsimd.dma_start(
                    out=qTpe_sb[:].rearrange("p (s q2) hh t -> p q2 s hh t",
                                             q2=2)[:, q2, :, h, :],
                    in_=a2aq_out[:, h, DN:, q2 * 128:(q2 + 1) * 128]
                        .rearrange("s d t -> d s t"))

        qabsT_sb = attn.tile([128, LC, NQB, HLOC, 128], BF, name="qabsT_sb")

        # ---------------- Stage 6: attention ------------------------------
        # (qabsT for each 512-token chunk is computed just before the
        # first query-block pair that needs it, so attention starts as
        # soon as the first chunk of the q AllToAll has been absorbed.)
        ps_o_cm = tc.tile_pool(name="ps_o", bufs=2, space="PSUM")
        ps_o = ps_o_cm.__enter__()
        pexp_cm = tc.tile_pool(name="pexp", bufs=3)
        pexp = pexp_cm.__enter__()
        onorm_cm = tc.tile_pool(name="onorm", bufs=3)
        onorm = onorm_cm.__enter__()
        wopool_cm = tc.tile_pool(name="wopool", bufs=4)
        wopool = wopool_cm.__enter__()
        outp_cm = tc.tile_pool(name="outp", bufs=4)
        outp = outp_cm.__enter__()
        ao_sb = attn.tile([128, HLOC, NQB, 128], BF, name="ao_sb")

        for qj in range(NQB // 2):
            qb0, qb1 = 2 * qj, 2 * qj + 1
            if qj % 2 == 0:
                tq = qj // 2
                for h in range(HLOC):
                    for lc in range(LC):
                        pqa = ps_mm.tile([128, 512], F32, tag="mm",
                                         name="pqa_ps")
                        nc.tensor.matmul(
                            pqa[:], wkc_sb[:, h, lc * 128:(lc + 1) * 128],
                            qT_sb[:, h, tq * 512:(tq + 1) * 512],
                            start=True, stop=True)
                        nc.vector.tensor_copy(
                            qabsT_sb[:, lc, tq * 4:(tq + 1) * 4, h, :],
                            pqa[:].rearrange("p (q t) -> p q t", q=4))
            pos = [ps_o.tile([128, HLOC, DVE_], F32, tag="po", name="po0"),
                   ps_o.tile([128, HLOC, DVE_], F32, tag="po", name="po1")]
            for kb in range(qb1 + 1):
                if kb <= qb0:
                    # both query blocks of the pair attend to this key block
                    psc = ps_mm.tile([128, 2, HLOC, 128], F32, tag="mm",
                                     name="psc2")
                    for lc in range(LC):
                        nc.tensor.matmul(
                            psc[:], kT_sb[:, lc, kb * 128:(kb + 1) * 128],
                            qabsT_sb[:, lc, qb0:qb1 + 1, :, :],
                            start=(lc == 0), stop=False)
                    nc.tensor.matmul(
                        psc[:], kTpe_sb[:, kb * 128:(kb + 1) * 128],
                        qTpe_sb[:, qb0:qb1 + 1, :, :],
                        start=False, stop=True)
                    p_bf = pexp.tile([128, 2, HLOC, 128], BF, tag="p_bf",
                                     name="p_bf")
                    nc.scalar.activation(p_bf[:], psc[:],
                                         mybir.ActivationFunctionType.Exp,
                                         scale=float(SCALE))
                    if kb == qb0:
                        nc.vector.tensor_mul(p_bf[:, 0], p_bf[:, 0], cmask_sb[:])
                    for qi in range(2):
                        for h in range(HLOC):
                            nc.tensor.matmul(
                                pos[qi][:, h, :], p_bf[:, qi, h, :],
                                v_sb[:, kb, h, :],
                                start=(kb == 0 and h == 0),
                                stop=(kb == (qb0 if qi == 0 else qb1)))
                else:
                    # kb == qb1: only the odd block (its diagonal)
                    psc1 = ps_mm.tile([128, HLOC, 128], F32, tag="mm",
                                      name="psc1")
                    for lc in range(LC):
                        nc.tensor.matmul(
                            psc1[:], kT_sb[:, lc, kb * 128:(kb + 1) * 128],
                            qabsT_sb[:, lc, qb1, :, :],
                            start=(lc == 0), stop=False)
                    nc.tensor.matmul(
                        psc1[:], kTpe_sb[:, kb * 128:(kb + 1) * 128],
                        qTpe_sb[:, qb1, :, :],
                        start=False, stop=True)
                    p1 = pexp.tile([128, HLOC, 128], BF, tag="p_bf", name="p1")
                    nc.scalar.activation(p1[:], psc1[:],
                                         mybir.ActivationFunctionType.Exp,
                                         scale=float(SCALE))
                    nc.vector.tensor_mul(p1[:], p1[:], cmask_sb[:])
                    for h in range(HLOC):
                        nc.tensor.matmul(
                            pos[1][:, h, :], p1[:, h, :],
                            v_sb[:, kb, h, :],
                            start=False, stop=True)
            for qi, qb in ((0, qb0), (1, qb1)):
                po = pos[qi]
                for h in range(HLOC):
                    rh = onorm.tile([128, 1], F32, tag="rh", name="rh")
                    nc.vector.reciprocal(rh[:], po[:, h, DV:DVE_])
                    ob = onorm.tile([128, DV], BF, tag="ob", name="ob")
                    nc.vector.tensor_scalar_mul(ob[:], in0=po[:, h, :DV],
                                                scalar1=rh[:])
                    pot = ps_tr.tile([128, 128], BF, tag="tr", name="pot_tr")
                    nc.tensor.transpose(pot[:], ob[:], ident[:])
                    nc.scalar.copy(ao_sb[:, h, qb, :], pot[:])

        for hh in range(HLOC):
            nc.gpsimd.dma_start(
                out=a2ao_in[:, hh, :, :].rearrange("s v (q2 t) -> v s q2 t", q2=2),
                in_=ao_sb[:, hh].rearrange("v (s q2) t -> v s q2 t", q2=2))
        nc.gpsimd.collective_compute(
            "AllToAll", mybir.AluOpType.bypass, replica_groups=RG,
            ins=[a2ao_in.opt()], outs=[a2ao_out.opt()])

        # ---------------- Stage 7: W_O ------------------------------------
        oT_sb = attn.tile([128, H, TLOC], BF, name="oT_sb")
        nc.gpsimd.dma_start(
            out=oT_sb[:].rearrange("p (s hh) t -> p s hh t", s=NCORES),
            in_=a2ao_out[:, :, :, :].rearrange("s hh v t -> v s hh t"))
        for ht in range(NHT):
            pso = [ps_mm.tile([128, 512], F32, tag="mm", name="wo_ps")
                   for _ in range(MCH)]
            for oc in range(2):
                wo_t = wopool.tile([128, 8 * 512], BF, tag="wo_t", name="wo_t")
                _oo = (ht * 2 + oc) * 8 * 128 * 512
                nc.sync.dma_start(
                    out=wo_t[:],
                    in_=wo[_oo:_oo + 8 * 128 * 512].rearrange("(p w) -> p w", p=128))
                for cc in range(8):
                    c = oc * 8 + cc
                    for m in range(MCH):
                        nc.tensor.matmul(pso[m][:],
                                         oT_sb[:, c, m * 128:(m + 1) * 128],
                                         wo_t[:, cc * 512:(cc + 1) * 512],
                                         start=(c == 0), stop=(c == H - 1))
            for m in range(MCH):
                ot = outp.tile([128, 512], F32, tag="ot", name="ot")
                if ht % 2 == 0:
                    nc.scalar.copy(ot[:], pso[m][:])
                else:
                    nc.vector.tensor_copy(ot[:], pso[m][:])
                nc.gpsimd.dma_start(
                    out=out[:, :].rearrange("(m p) d -> p m d", p=128)[
                        :, m, ht * 512:(ht + 1) * 512],
                    in_=ot[:])

        for p in (outp_cm, wopool_cm, onorm_cm, pexp_cm, ps_o_cm, attn_cm,
                  attnkv_cm, ps_tr_cm, ps_mm_cm, dram_cm, consts_cm):
            p.__exit__(None, None, None)

    nc.finalize()
    return nc


def _to_bf16(a):
    return np.asarray(a, dtype=np.float32).astype(ml_dtypes.bfloat16)


def _pack_rhs(w, col0, ncols, k0, nk):
    """[128, nk*ncols] p-major contiguous block of w[k0*128:(k0+nk)*128,
    col0:col0+ncols] with the nk contraction chunks along the free dim."""
    blk = w[k0 * 128:(k0 + nk) * 128, col0:col0 + ncols].reshape(nk, 128, ncols)
    return np.ascontiguousarray(blk.transpose(1, 0, 2)).reshape(-1)


def _prep_in_maps(positions, hidden_states, w_fused, w_qb, w_kvb, w_o,
                  qa_ln_w, kva_ln_w):
    positions = np.asarray(positions)
    hidden_states = np.asarray(hidden_states, dtype=np.float32)
    w_fused = np.asarray(w_fused, dtype=np.float32)
    w_qb = np.asarray(w_qb, dtype=np.float32)
    w_kvb = np.asarray(w_kvb, dtype=np.float32)
    w_o = np.asarray(w_o, dtype=np.float32)
    qa_ln_w = np.asarray(qa_ln_w, dtype=np.float32)
    kva_ln_w = np.asarray(kva_ln_w, dtype=np.float32)

    inv_freq = 1.0 / (THETA ** (np.arange(0, DR, 2, dtype=np.float32) / DR))
    freqs = positions.astype(np.float32)[:, None] * inv_freq[None, :]
    cs_full = np.concatenate([np.cos(freqs), np.sin(freqs)], axis=1)  # [T, 64]

    wqb_folded = qa_ln_w[:, None] * w_qb
    wkvb_r = w_kvb.reshape(KVL, H, DN + DV)

    wf_bf = _to_bf16(np.concatenate(
        [_pack_rhs(w_fused, g0, gw, hh * (KD // nh), KD // nh)
         for g0, gw, nh in AGROUPS for hh in range(nh)]))
    wqb_bf = _to_bf16(np.concatenate(
        [_pack_rhs(wqb_folded, ng * 512, 512, 0, QKD) for ng in range(NGQ)]))
    wo_bf = _to_bf16(np.concatenate(
        [_pack_rhs(w_o, ht * 512, 512, oc * 8, 8)
         for ht in range(NHT) for oc in range(2)]))

    tri = np.triu(np.ones((128, 128), np.float32))
    cmask = _to_bf16(np.repeat(tri[:, None, :], HLOC, axis=1))

    in_maps = []
    for c in range(NCORES):
        tok = slice(c * TLOC, (c + 1) * TLOC)
        heads = [HLOC * c + i for i in range(HLOC)]
        wkcT = np.stack([(wkvb_r[:, h, :DN] * kva_ln_w[:, None]).T for h in heads])
        wvc = np.concatenate(
            [wkvb_r[:, h, DN:] * kva_ln_w[:, None] for h in heads], axis=1)
        hT_full = np.ascontiguousarray(
            hidden_states[tok].T.reshape(KD, 128, TLOC))
        hT_packed = np.concatenate(
            [np.ascontiguousarray(
                hT_full[hh * (KD // 2):(hh + 1) * (KD // 2)]
                .transpose(1, 0, 2)).reshape(-1) for hh in range(2)])
        in_maps.append({
            "hT": _to_bf16(hT_packed),
            "wf": wf_bf,
            "wqb": wqb_bf,
            "cs": np.ascontiguousarray(cs_full[tok]),
            "wkcT": _to_bf16(np.ascontiguousarray(wkcT)),
            "wvc": _to_bf16(np.ascontiguousarray(wvc)),
            "wo": wo_bf,
            "cmask": cmask,
        })
    return in_maps


def kernel(**inputs):
    global _NC_CACHE, _last_in_maps
    in_maps = _prep_in_maps(**inputs)
    _last_in_maps = in_maps
    if _NC_CACHE is None:
        _NC_CACHE = build_nc()

    res = run_bass_kernel_spmd(_NC_CACHE, in_maps, core_ids=list(range(NCORES)))
    return np.concatenate([np.asarray(res.results[c]["out"], dtype=np.float32)
                           for c in range(NCORES)], axis=0)


if __name__ == "__main__":
    build_nc()
    print("build ok")


# revision 17
# speedup vs baseline: 1.1027x; 1.1027x over previous
"""DeepseekV2 MLA attention on 8 Trainium2 NeuronCores.

Sharding (uniform SPMD, no control divergence):
- A-projection, q-up-projection, final W_O: token-sharded (core c owns
  tokens [256c, 256c+256)).
- Attention (absorbed MLA over the compressed KV latent): head-sharded
  (core c owns heads {2c, 2c+1}).
- Three collectives connect the shardings: AllGather of the kv latent
  (feature-major, bf16), AllToAll of q^T (token->head resharding),
  AllToAll of normalized o^T (head->token resharding).

All matmuls run in bf16 with fp32 PSUM accumulation. RMSNorm weights are
folded into the adjacent weight matrices on the host. Softmax runs
unnormalized (logits are small by construction: std ~0.7) with the
denominator obtained by appending a ones-column to V; normalization is a
per-partition scale on the token-major attention output.

v2: all weight streams are host-packed into large contiguous p-major
blocks so each DMA moves 0.6-2.6 MB (the sync queue serializes DMA
issue at ~600ns each, capping many-small-DMA phases at ~100-200 GB/s).
RMSNorm/rope consume the A-proj PSUM directly; final output writes go
out on the gpsimd queue so they cannot stall w_o weight prefetch.
"""

import os
import sys

for _p in ("/opt/trn_rl_repo", "/root/.axon_site", "/root/.axon_site/_ro/trn_rl_repo",
           "/root/.axon_site/_ro/pypackages"):
    if os.path.isdir(_p) and _p not in sys.path:
        sys.path.insert(0, _p)

import numpy as np
import ml_dtypes

import concourse.bass as bass
import concourse.tile as tile
from concourse import bacc, mybir
from concourse.bass_utils import run_bass_kernel_spmd
from concourse.masks import make_identity

# Problem constants (hardcoded per contract)
T, HID, H = 2048, 5120, 16
DN, DR, DV = 128, 64, 128
QL, KVL = 1536, 512
EPS = 1e-6
THETA = 10000.0
SCALE = (DN + DR) ** -0.5

NCORES = 8
TLOC = T // NCORES          # 256 tokens per core
HLOC = H // NCORES          # 2 heads per core
MCH = TLOC // 128           # 2 token chunks of 128
KD = HID // 128             # 40 contraction chunks for A-proj
QKD = QL // 128             # 12 contraction chunks for q-up
LC = KVL // 128             # 4 latent chunks
NQB = T // 128              # 16 query/key blocks of 128
DQK = DN + DR               # 192
DVE_ = DV + 1               # 129: extra ones-column for softmax denominator
NGQ = H * DQK // 512        # 6 q-up output groups of 512
NHT = HID // 512            # 10 W_O output groups of 512

# A-proj column groups: (g0, gw, n_halves) in processing order
# (kv latent first so the AllGather chain launches early, then rope
# cols, then the three q-latent groups).
AGROUPS = [(QL, KVL, 4), (QL + KVL, DR, 1), (0, 512, 2), (512, 512, 2),
           (1024, 512, 2)]

BF = mybir.dt.bfloat16
F32 = mybir.dt.float32

_NC_CACHE = None
_last_in_maps = None


def _rope_pair(nc, pool, x_pairs, cos, sin, out_pairs, shape, dt=F32):
    """Interleaved rope: out1 = x1*cos - x2*sin ; out2 = x2*cos + x1*sin."""
    x1, x2 = x_pairs[:, 0], x_pairs[:, 1]
    o1, o2 = out_pairs[:, 0], out_pairs[:, 1]
    tm1 = pool.tile([128] + shape, dt, tag="rope_tm1", name="rope_tm1")
    tm2 = pool.tile([128] + shape, dt, tag="rope_tm2", name="rope_tm2")
    tm3 = pool.tile([128] + shape, dt, tag="rope_tm3", name="rope_tm3")
    nc.vector.tensor_mul(tm1[:], x1, cos)
    nc.vector.tensor_mul(tm2[:], x2, sin)
    nc.vector.tensor_mul(tm3[:], x1, sin)
    nc.vector.tensor_sub(o1, tm1[:], tm2[:])
    nc.vector.tensor_mul(tm1[:], x2, cos)
    nc.vector.tensor_add(o2, tm1[:], tm3[:])


def build_nc():
    nc = bacc.Bacc("TRN2", target_bir_lowering=False, debug=False,
                   num_devices=NCORES)

    hT = nc.dram_tensor("hT", [KD * 128 * TLOC], BF, kind="ExternalInput")
    wf = nc.dram_tensor("wf", [HID * (QL + KVL + DR)], BF, kind="ExternalInput")
    wqb = nc.dram_tensor("wqb", [QL * H * DQK], BF, kind="ExternalInput")
    cs = nc.dram_tensor("cs", [TLOC, DR], F32, kind="ExternalInput")
    wkcT = nc.dram_tensor("wkcT", [HLOC, DN, KVL], BF, kind="ExternalInput")
    wvc = nc.dram_tensor("wvc", [KVL, HLOC * DV], BF, kind="ExternalInput")
    wo = nc.dram_tensor("wo", [H * DV * HID], BF, kind="ExternalInput")
    cmask = nc.dram_tensor("cmask", [128, HLOC, 128], BF, kind="ExternalInput")
    out = nc.dram_tensor("out", [TLOC, HID], F32, kind="ExternalOutput")

    RG = [list(range(NCORES))]

    with tile.TileContext(nc) as tc:
        consts_cm = tc.tile_pool(name="consts", bufs=1)
        consts = consts_cm.__enter__()
        dram_cm = tc.tile_pool(name="dram", bufs=1, space="DRAM")
        dram = dram_cm.__enter__()
        ps_mm_cm = tc.tile_pool(name="ps_mm", bufs=4, space="PSUM")
        ps_mm = ps_mm_cm.__enter__()
        ps_tr_cm = tc.tile_pool(name="ps_tr", bufs=2, space="PSUM")
        ps_tr = ps_tr_cm.__enter__()
        attnkv_cm = tc.tile_pool(name="attnkv", bufs=1)
        attnkv = attnkv_cm.__enter__()

        ident = consts.tile([128, 128], BF, name="ident")
        make_identity(nc, ident[:])
        cmask_sb = consts.tile([128, HLOC, 128], BF, name="cmask_sb")
        wkc_sb = consts.tile([128, HLOC, KVL], BF, name="wkc_sb")
        eps_sb = consts.tile([128, 1], F32, name="eps_sb")
        nc.vector.memset(eps_sb[:], float(EPS))
        wvc_sb = consts.tile([128, LC, HLOC * DV], BF, name="wvc_sb")

        # collective DRAM tiles
        ag_in = dram.tile([KVL + DR, TLOC], BF, name="ag_in")
        ag_out = dram.tile([NCORES, KVL + DR, TLOC], BF, addr_space="Shared",
                           name="ag_out")
        a2aq_in = dram.tile([NCORES, HLOC, DQK, TLOC], BF, name="a2aq_in")
        a2aq_out = dram.tile([NCORES, HLOC, DQK, TLOC], BF, name="a2aq_out")
        a2ao_in = dram.tile([NCORES, HLOC, DV, TLOC], BF, name="a2ao_in")
        a2ao_out = dram.tile([NCORES, HLOC, DV, TLOC], BF, name="a2ao_out")

        # ---------------- Stages 1-3: token-sharded projections -----------
        early_cm = tc.tile_pool(name="early", bufs=1)
        early = early_cm.__enter__()
        tmp_cm = tc.tile_pool(name="tmp", bufs=1)
        tmp = tmp_cm.__enter__()
        wfpool_cm = tc.tile_pool(name="wfpool", bufs=2)
        wfpool = wfpool_cm.__enter__()

        hT_sb = early.tile([128, KD, TLOC], BF, name="hT_sb")
        KH = KD // 2
        for h in range(2):
            nc.sync.dma_start(
                out=hT_sb[:, h * KH:(h + 1) * KH, :],
                in_=hT[h * 128 * KH * TLOC:(h + 1) * 128 * KH * TLOC]
                    .rearrange("(p x) -> p x", p=128))
        cs_sb = early.tile([128, MCH, DR], F32, name="cs_sb")
        nc.sync.dma_start(out=cs_sb[:],
                          in_=cs[:, :].rearrange("(m p) d -> p m d", p=128))

        wf_off = [0]

        def aproj_group(g0, gw, nh):
            """A-proj over one column group; returns per-m psum tiles."""
            kper = KD // nh
            pss = [ps_mm.tile([128, gw], F32, tag="mm", name="aproj_ps")
                   for _ in range(MCH)]
            for hh in range(nh):
                wf_t = wfpool.tile([128, kper * gw], BF, tag="wf_t", name="wf_t")
                nc.sync.dma_start(
                    out=wf_t[:],
                    in_=wf[wf_off[0]:wf_off[0] + 128 * kper * gw]
                        .rearrange("(p w) -> p w", p=128))
                wf_off[0] += 128 * kper * gw
                for kk in range(kper):
                    k = hh * kper + kk
                    for m in range(MCH):
                        nc.tensor.matmul(
                            pss[m][:], hT_sb[:, k, m * 128:(m + 1) * 128],
                            wf_t[:, kk * gw:(kk + 1) * gw],
                            start=(k == 0), stop=(k == KD - 1))
            return pss

        pss_kv = aproj_group(*AGROUPS[0])
        pss_pe = aproj_group(*AGROUPS[1])

        # ---------------- Stage 2: kv latent + rope + AllGather -----------
        kvlat_bf = early.tile([128, MCH, KVL], BF, name="kvlat_bf")
        kpe_bf = early.tile([128, MCH, DR], BF, name="kpe_bf")
        agin_sb = early.tile([128, LC, MCH, 128], BF, name="agin_sb")
        agpe_sb = early.tile([64, MCH, 128], BF, name="agpe_sb")

        for m in range(MCH):
            sq = tmp.tile([128, KVL], F32, tag="sq_kv", name="sq_kv")
            ssum = tmp.tile([128, 1], F32, tag="ssum_kv", name="ssum_kv")
            nc.scalar.activation(sq[:], pss_kv[m][:],
                                 mybir.ActivationFunctionType.Square,
                                 accum_out=ssum[:])
            rstd = tmp.tile([128, 1], F32, tag="rstd_kv", name="rstd_kv")
            nc.scalar.activation(rstd[:], ssum[:],
                                 mybir.ActivationFunctionType.Sqrt,
                                 bias=eps_sb[:], scale=1.0 / KVL)
            rinv = tmp.tile([128, 1], F32, tag="rinv_kv", name="rinv_kv")
            nc.vector.reciprocal(rinv[:], rstd[:])
            nc.vector.tensor_scalar_mul(kvlat_bf[:, m], in0=pss_kv[m][:],
                                        scalar1=rinv[:])
            kv_pairs = pss_pe[m][:].rearrange("p (i two) -> p two i", two=2)
            out_pairs = kpe_bf[:, m].rearrange("p (i two) -> p two i", two=2)
            _rope_pair(nc, tmp, kv_pairs,
                       cs_sb[:, m, :DR // 2], cs_sb[:, m, DR // 2:],
                       out_pairs, [DR // 2])
            for lc in range(LC):
                pt = ps_tr.tile([128, 128], BF, tag="tr", name="pt_tr")
                nc.tensor.transpose(pt[:], kvlat_bf[:, m, lc * 128:(lc + 1) * 128],
                                    ident[:])
                nc.vector.tensor_copy(agin_sb[:, lc, m, :], pt[:])
            ptp = ps_tr.tile([64, 128], BF, tag="tr", name="ptp_tr")
            nc.tensor.transpose(ptp[:], kpe_bf[:, m], ident[:])
            nc.vector.tensor_copy(agpe_sb[:, m, :], ptp[:])

        nc.gpsimd.dma_start(
            out=ag_in[:KVL, :].rearrange("(c p) m -> p c m", p=128)
                              .rearrange("p c (m t) -> p c m t", m=MCH),
            in_=agin_sb[:])
        nc.gpsimd.dma_start(
            out=ag_in[KVL:, :].rearrange("p (m t) -> p m t", m=MCH),
            in_=agpe_sb[:])
        nc.gpsimd.collective_compute(
            "AllGather", mybir.AluOpType.bypass, replica_groups=RG,
            ins=[ag_in.opt()], outs=[ag_out.opt()])

        # attention-phase constants: loaded off the startup critical path
        nc.sync.dma_start(out=cmask_sb[:], in_=cmask[:, :, :])
        nc.sync.dma_start(out=wkc_sb[:], in_=wkcT[:, :, :].rearrange("h d l -> d h l"))
        nc.sync.dma_start(out=wvc_sb[:],
                          in_=wvc[:, :].rearrange("(c p) v -> p c v", p=128))

        # ---------------- Stage 3: q path ---------------------------------
        qkv_q = early.tile([128, MCH, QL], F32, name="qkv_q")
        qg_ss = []
        for gi in range(3):
            pss = aproj_group(*AGROUPS[2 + gi])
            for m in range(MCH):
                sq2 = tmp.tile([128, 512], F32, tag="sq_q", name="sq_q")
                ssum2 = tmp.tile([128, 1], F32, tag=f"ssum_q{gi}m{m}",
                                 name="ssum_q")
                nc.scalar.activation(sq2[:], pss[m][:],
                                     mybir.ActivationFunctionType.Square,
                                     accum_out=ssum2[:])
                nc.vector.tensor_copy(qkv_q[:, m, gi * 512:(gi + 1) * 512],
                                      pss[m][:])
                qg_ss.append(ssum2)
        wfpool_cm.__exit__(None, None, None)

        qan_bf = early.tile([128, MCH, QL], BF, name="qan_bf")
        for m in range(MCH):
            st = tmp.tile([128, 1], F32, tag=f"stot{m}", name="stot")
            nc.vector.tensor_add(st[:], qg_ss[m][:], qg_ss[2 + m][:])
            nc.vector.tensor_add(st[:], st[:], qg_ss[4 + m][:])
            rstd2 = tmp.tile([128, 1], F32, tag="rstd_q", name="rstd_q")
            nc.scalar.activation(rstd2[:], st[:],
                                 mybir.ActivationFunctionType.Sqrt,
                                 bias=eps_sb[:], scale=1.0 / QL)
            rinv2 = tmp.tile([128, 1], F32, tag="rinv_q", name="rinv_q")
            nc.vector.reciprocal(rinv2[:], rstd2[:])
            nc.vector.tensor_scalar_mul(qan_bf[:, m], in0=qkv_q[:, m],
                                        scalar1=rinv2[:])

        qanT_sb = early.tile([128, QKD, TLOC], BF, name="qanT_sb")
        for m in range(MCH):
            for kc in range(QKD):
                pt = ps_tr.tile([128, 128], BF, tag="tr", name="pt_tr")
                nc.tensor.transpose(pt[:], qan_bf[:, m, kc * 128:(kc + 1) * 128],
                                    ident[:])
                nc.vector.tensor_copy(qanT_sb[:, kc, m * 128:(m + 1) * 128], pt[:])

        # q-up: one 1.57MB weight DMA per 512-col output group
        q_bf = early.tile([128, MCH, H, DQK], BF, name="q_bf")
        wqbpool_cm = tc.tile_pool(name="wqbpool", bufs=2)
        wqbpool = wqbpool_cm.__enter__()
        for ng in range(NGQ):
            wqb_t = wqbpool.tile([128, QKD * 512], BF, tag="wqb_t", name="wqb_t")
            _qo = ng * QKD * 128 * 512
            nc.sync.dma_start(
                out=wqb_t[:],
                in_=wqb[_qo:_qo + QKD * 128 * 512].rearrange("(p w) -> p w", p=128))
            psq = [ps_mm.tile([128, 512], F32, tag="mm", name="qup_ps")
                   for _ in range(MCH)]
            for kc in range(QKD):
                for m in range(MCH):
                    nc.tensor.matmul(psq[m][:], qanT_sb[:, kc, m * 128:(m + 1) * 128],
                                     wqb_t[:, kc * 512:(kc + 1) * 512],
                                     start=(kc == 0), stop=(kc == QKD - 1))
            for m in range(MCH):
                dst = q_bf[:, m].rearrange("p h d -> p (h d)")[:,
                                                              ng * 512:(ng + 1) * 512]
                if ng % 2 == 0:
                    nc.scalar.copy(dst, psq[m][:])
                else:
                    nc.vector.tensor_copy(dst, psq[m][:])
        wqbpool_cm.__exit__(None, None, None)

        # K/V prep for attention: depends only on the AllGather. Loaded on
        # the gpsimd queue (idle between the AG and the q AllToAll input;
        # on the sync queue the scheduler hoists these AG-blocked loads
        # ahead of the wf_q weight streams and stalls the whole q path).
        kT_sb = attnkv.tile([128, LC, T], BF, name="kT_sb")
        for lc in range(LC):
            nc.gpsimd.dma_start(
                out=kT_sb[:, lc].rearrange("p (s t) -> p s t", s=NCORES),
                in_=ag_out[:, lc * 128:(lc + 1) * 128, :].rearrange("s p t -> p s t"))
        kTpe_sb = attnkv.tile([64, T], BF, name="kTpe_sb")
        nc.gpsimd.dma_start(
            out=kTpe_sb[:].rearrange("p (s t) -> p s t", s=NCORES),
            in_=ag_out[:, KVL:, :].rearrange("s p t -> p s t"))
        v_sb = attnkv.tile([128, NQB, HLOC, DVE_], BF, name="v_sb")
        nc.vector.memset(v_sb[:, :, :, DV:], 1.0)
        for tcb in range(NQB):
            pv = ps_mm.tile([128, HLOC * DV], F32, tag="mm", name="pv_ps")
            for lc in range(LC):
                nc.tensor.matmul(pv[:], kT_sb[:, lc, tcb * 128:(tcb + 1) * 128],
                                 wvc_sb[:, lc, :], start=(lc == 0),
                                 stop=(lc == LC - 1))
            nc.vector.tensor_copy(
                v_sb[:, tcb, :, :DV],
                pv[:].rearrange("p (h v) -> p h v", h=HLOC))

        # rope on q_pe: batched over all heads, bf16 in-place on q_bf
        # (cos/sin broadcast across heads via a zero-stride AP dim)
        cs_bf = early.tile([128, MCH, DR], BF, name="cs_bf")
        nc.vector.tensor_copy(cs_bf[:], cs_sb[:])

        def _bcast_h(apx):
            return bass.AP(tensor=apx.tensor, offset=apx.offset,
                           ap=[apx.ap[0], [0, H], apx.ap[-1]])

        for m in range(MCH):
            q_pairs = q_bf[:, m, :, DN:].rearrange("p h (i two) -> p two h i", two=2)
            _rope_pair(nc, tmp, q_pairs,
                       _bcast_h(cs_bf[:, m, :DR // 2]),
                       _bcast_h(cs_bf[:, m, DR // 2:]),
                       q_pairs, [H, DR // 2], dt=BF)

        aq0_sb = early.tile([128, H, MCH, 128], BF, name="aq0_sb")
        aq1_sb = early.tile([64, H, MCH, 128], BF, name="aq1_sb")
        for m in range(MCH):
            for h in range(H):
                pt0 = ps_tr.tile([128, 128], BF, tag="tr", name="pt0_tr")
                nc.tensor.transpose(pt0[:], q_bf[:, m, h, :DN], ident[:])
                nc.vector.tensor_copy(aq0_sb[:, h, m, :], pt0[:])
                pt1 = ps_tr.tile([64, 128], BF, tag="tr", name="pt1_tr")
                nc.tensor.transpose(pt1[:], q_bf[:, m, h, DN:], ident[:])
                nc.vector.tensor_copy(aq1_sb[:, h, m, :], pt1[:])
        _aqv = a2aq_in[:, :, :, :].rearrange("s hh d t -> (s hh) d t") \
                                  .rearrange("h d (m t) -> d h m t", m=MCH)
        nc.gpsimd.dma_start(out=_aqv[:DN], in_=aq0_sb[:])
        nc.gpsimd.dma_start(out=_aqv[DN:], in_=aq1_sb[:])
        nc.gpsimd.collective_compute(
            "AllToAll", mybir.AluOpType.bypass, replica_groups=RG,
            ins=[a2aq_in.opt()], outs=[a2aq_out.opt()])

        tmp_cm.__exit__(None, None, None)
        early_cm.__exit__(None, None, None)

        attn_cm = tc.tile_pool(name="attn", bufs=1)
        attn = attn_cm.__enter__()

        # ---------------- Stage 5: q^T for my heads -----------------------
        qT_sb = attn.tile([128, HLOC, T], BF, name="qT_sb")
        qTpe_sb = attn.tile([64, NQB, HLOC, 128], BF, name="qTpe_sb")
        for h in range(HLOC):
            nc.gpsimd.dma_start(
                out=qT_sb[:, h].rearrange("p (s t) -> p s t", s=NCORES),
                in_=a2aq_out[:, h, :DN, :].rearrange("s d t -> d s t"))
            for q2 in range(2):
                nc.gpsimd.dma_start(
                    out=qTpe_sb[:].rearrange("p (s q2) hh t -> p q2 s hh t",
                                             q2=2)[:, q2, :, h, :],
                    in_=a2aq_out[:, h, DN:, q2 * 128:(q2 + 1) * 128]
                        .rearrange("s d t -> d s t"))

        qabsT_sb = attn.tile([128, LC, NQB, HLOC, 128], BF, name="qabsT_sb")

        # ---------------- Stage 6: attention ------------------------------
        # (qabsT for each 512-token chunk is computed just before the
        # first query-block pair that needs it, so attention starts as
        # soon as the first chunk of the q AllToAll has been absorbed.)
        ps_o_cm = tc.tile_pool(name="ps_o", bufs=2, space="PSUM")
        ps_o = ps_o_cm.__enter__()
        pexp_cm = tc.tile_pool(name="pexp", bufs=3)
        pexp = pexp_cm.__enter__()
        onorm_cm = tc.tile_pool(name="onorm", bufs=3)
        onorm = onorm_cm.__enter__()
        wopool_cm = tc.tile_pool(name="wopool", bufs=4)
        wopool = wopool_cm.__enter__()
        outp_cm = tc.tile_pool(name="outp", bufs=4)
        outp = outp_cm.__enter__()
        ao_sb = attn.tile([128, HLOC, NQB, 128], BF, name="ao_sb")

        for qj in range(NQB // 2):
            qb0, qb1 = 2 * qj, 2 * qj + 1
            if qj % 2 == 0:
                tq = qj // 2
                for h in range(HLOC):
                    for lc in range(LC):
                        pqa = ps_mm.tile([128, 512], F32, tag="mm",
                                         name="pqa_ps")
                        nc.tensor.matmul(
                            pqa[:], wkc_sb[:, h, lc * 128:(lc + 1) * 128],
                            qT_sb[:, h, tq * 512:(tq + 1) * 512],
                            start=True, stop=True)
                        nc.vector.tensor_copy(
                            qabsT_sb[:, lc, tq * 4:(tq + 1) * 4, h, :],
                            pqa[:].rearrange("p (q t) -> p q t", q=4))
            pos = [ps_o.tile([128, HLOC, DVE_], F32, tag="po", name="po0"),
                   ps_o.tile([128, HLOC, DVE_], F32, tag="po", name="po1")]
            for kb in range(qb1 + 1):
                if kb <= qb0:
                    # both query blocks of the pair attend to this key block
                    psc = ps_mm.tile([128, 2, HLOC, 128], F32, tag="mm",
                                     name="psc2")
                    for lc in range(LC):
                        nc.tensor.matmul(
                            psc[:], kT_sb[:, lc, kb * 128:(kb + 1) * 128],
                            qabsT_sb[:, lc, qb0:qb1 + 1, :, :],
                            start=(lc == 0), stop=False)
                    nc.tensor.matmul(
                        psc[:], kTpe_sb[:, kb * 128:(kb + 1) * 128],
                        qTpe_sb[:, qb0:qb1 + 1, :, :],
                        start=False, stop=True)
                    p_bf = pexp.tile([128, 2, HLOC, 128], BF, tag="p_bf",
                                     name="p_bf")
                    nc.scalar.activation(p_bf[:], psc[:],
                                         mybir.ActivationFunctionType.Exp,
                                         scale=float(SCALE))
                    if kb == qb0:
                        nc.vector.tensor_mul(p_bf[:, 0], p_bf[:, 0], cmask_sb[:])
                    for qi in range(2):
                        for h in range(HLOC):
                            nc.tensor.matmul(
                                pos[qi][:, h, :], p_bf[:, qi, h, :],
                                v_sb[:, kb, h, :],
                                start=(kb == 0 and h == 0),
                                stop=(kb == (qb0 if qi == 0 else qb1)))
                else:
                    # kb == qb1: only the odd block (its diagonal)
                    psc1 = ps_mm.tile([128, HLOC, 128], F32, tag="mm",
                                      name="psc1")
                    for lc in range(LC):
                        nc.tensor.matmul(
                            psc1[:], kT_sb[:, lc, kb * 128:(kb + 1) * 128],
                            qabsT_sb[:, lc, qb1, :, :],
                            start=(lc == 0), stop=False)
                    nc.tensor.matmul(
                        psc1[:], kTpe_sb[:, kb * 128:(kb + 1) * 128],
                        qTpe_sb[:, qb1, :, :],
                        start=False, stop=True)
                    p1 = pexp.tile([128, HLOC, 128], BF, tag="p_bf", name="p1")
                    nc.scalar.activation(p1[:], psc1[:],
                                         mybir.ActivationFunctionType.Exp,
                                         scale=float(SCALE))
                    nc.vector.tensor_mul(p1[:], p1[:], cmask_sb[:])
                    for h in range(HLOC):
                        nc.tensor.matmul(
                            pos[1][:, h, :], p1[:, h, :],
                            v_sb[:, kb, h, :],
                            start=False, stop=True)
            for qi, qb in ((0, qb0), (1, qb1)):
                po = pos[qi]
                for h in range(HLOC):
                    rh = onorm.tile([128, 1], F32, tag="rh", name="rh")
                    nc.vector.reciprocal(rh[:], po[:, h, DV:DVE_])
                    ob = onorm.tile([128, DV], BF, tag="ob", name="ob")
                    nc.vector.tensor_scalar_mul(ob[:], in0=po[:, h, :DV],
                                                scalar1=rh[:])
                    pot = ps_tr.tile([128, 128], BF, tag="tr", name="pot_tr")
                    nc.tensor.transpose(pot[:], ob[:], ident[:])
                    nc.scalar.copy(ao_sb[:, h, qb, :], pot[:])

        for hh in range(HLOC):
            nc.gpsimd.dma_start(
                out=a2ao_in[:, hh, :, :].rearrange("s v (q2 t) -> v s q2 t", q2=2),
                in_=ao_sb[:, hh].rearrange("v (s q2) t -> v s q2 t", q2=2))
        nc.gpsimd.collective_compute(
            "AllToAll", mybir.AluOpType.bypass, replica_groups=RG,
            ins=[a2ao_in.opt()], outs=[a2ao_out.opt()])

        # ---------------- Stage 7: W_O ------------------------------------
        oT_sb = attn.tile([128, H, TLOC], BF, name="oT_sb")
        nc.gpsimd.dma_start(
            out=oT_sb[:].rearrange("p (s hh) t -> p s hh t", s=NCORES),
            in_=a2ao_out[:, :, :, :].rearrange("s hh v t -> v s hh t"))
        for ht in range(NHT):
            pso = [ps_mm.tile([128, 512], F32, tag="mm", name="wo_ps")
                   for _ in range(MCH)]
            for oc in range(2):
                wo_t = wopool.tile([128, 8 * 512], BF, tag="wo_t", name="wo_t")
                _oo = (ht * 2 + oc) * 8 * 128 * 512
                nc.sync.dma_start(
                    out=wo_t[:],
                    in_=wo[_oo:_oo + 8 * 128 * 512].rearrange("(p w) -> p w", p=128))
                for cc in range(8):
                    c = oc * 8 + cc
                    for m in range(MCH):
                        nc.tensor.matmul(pso[m][:],
                                         oT_sb[:, c, m * 128:(m + 1) * 128],
                                         wo_t[:, cc * 512:(cc + 1) * 512],
                                         start=(c == 0), stop=(c == H - 1))
            for m in range(MCH):
                ot = outp.tile([128, 512], F32, tag="ot", name="ot")
                if ht % 2 == 0:
                    nc.scalar.copy(ot[:], pso[m][:])
                else:
                    nc.vector.tensor_copy(ot[:], pso[m][:])
                nc.gpsimd.dma_start(
                    out=out[:, :].rearrange("(m p) d -> p m d", p=128)[
                        :, m, ht * 512:(ht + 1) * 512],
                    in_=ot[:])

        for p in (outp_cm, wopool_cm, onorm_cm, pexp_cm, ps_o_cm, attn_cm,
                  attnkv_cm, ps_tr_cm, ps_mm_cm, dram_cm, consts_cm):
            p.__exit__(None, None, None)

    nc.finalize()
    return nc


def _to_bf16(a):
    return np.asarray(a, dtype=np.float32).astype(ml_dtypes.bfloat16)


def _pack_rhs(w, col0, ncols, k0, nk):
    """[128, nk*ncols] p-major contiguous block of w[k0*128:(k0+nk)*128,
    col0:col0+ncols] with the nk contraction chunks along the free dim."""
    blk = w[k0 * 128:(k0 + nk) * 128, col0:col0 + ncols].reshape(nk, 128, ncols)
    return np.ascontiguousarray(blk.transpose(1, 0, 2)).reshape(-1)


def _prep_in_maps(positions, hidden_states, w_fused, w_qb, w_kvb, w_o,
                  qa_ln_w, kva_ln_w):
    positions = np.asarray(positions)
    hidden_states = np.asarray(hidden_states, dtype=np.float32)
    w_fused = np.asarray(w_fused, dtype=np.float32)
    w_qb = np.asarray(w_qb, dtype=np.float32)
    w_kvb = np.asarray(w_kvb, dtype=np.float32)
    w_o = np.asarray(w_o, dtype=np.float32)
    qa_ln_w = np.asarray(qa_ln_w, dtype=np.float32)
    kva_ln_w = np.asarray(kva_ln_w, dtype=np.float32)

    inv_freq = 1.0 / (THETA ** (np.arange(0, DR, 2, dtype=np.float32) / DR))
    freqs = positions.astype(np.float32)[:, None] * inv_freq[None, :]
    cs_full = np.concatenate([np.cos(freqs), np.sin(freqs)], axis=1)  # [T, 64]

    wqb_folded = qa_ln_w[:, None] * w_qb
    wkvb_r = w_kvb.reshape(KVL, H, DN + DV)

    wf_bf = _to_bf16(np.concatenate(
        [_pack_rhs(w_fused, g0, gw, hh * (KD // nh), KD // nh)
         for g0, gw, nh in AGROUPS for hh in range(nh)]))
    wqb_bf = _to_bf16(np.concatenate(
        [_pack_rhs(wqb_folded, ng * 512, 512, 0, QKD) for ng in range(NGQ)]))
    wo_bf = _to_bf16(np.concatenate(
        [_pack_rhs(w_o, ht * 512, 512, oc * 8, 8)
         for ht in range(NHT) for oc in range(2)]))

    tri = np.triu(np.ones((128, 128), np.float32))
    cmask = _to_bf16(np.repeat(tri[:, None, :], HLOC, axis=1))

    in_maps = []
    for c in range(NCORES):
        tok = slice(c * TLOC, (c + 1) * TLOC)
        heads = [HLOC * c + i for i in range(HLOC)]
        wkcT = np.stack([(wkvb_r[:, h, :DN] * kva_ln_w[:, None]).T for h in heads])
        wvc = np.concatenate(
            [wkvb_r[:, h, DN:] * kva_ln_w[:, None] for h in heads], axis=1)
        hT_full = np.ascontiguousarray(
            hidden_states[tok].T.reshape(KD, 128, TLOC))
        hT_packed = np.concatenate(
            [np.ascontiguousarray(
                hT_full[hh * (KD // 2):(hh + 1) * (KD // 2)]
                .transpose(1, 0, 2)).reshape(-1) for hh in range(2)])
        in_maps.append({
            "hT": _to_bf16(hT_packed),
            "wf": wf_bf,
            "wqb": wqb_bf,
            "cs": np.ascontiguousarray(cs_full[tok]),
            "wkcT": _to_bf16(np.ascontiguousarray(wkcT)),
            "wvc": _to_bf16(np.ascontiguousarray(wvc)),
            "wo": wo_bf,
            "cmask": cmask,
        })
    return in_maps


def kernel(**inputs):
    global _NC_CACHE, _last_in_maps
    in_maps = _prep_in_maps(**inputs)
    _last_in_maps = in_maps
    if _NC_CACHE is None:
        _NC_CACHE = build_nc()

    res = run_bass_kernel_spmd(_NC_CACHE, in_maps, core_ids=list(range(NCORES)))
    return np.concatenate([np.asarray(res.results[c]["out"], dtype=np.float32)
                           for c in range(NCORES)], axis=0)


if __name__ == "__main__":
    build_nc()
    print("build ok")
